# revision 5
# baseline (speedup 1.0000x reference)
"""CrossModalCoherenceMetric Trainium2 kernel.

Full-input contract: kernel(**inputs) takes the unsharded inputs
(step_embeddings (8192,1024) f32, modal_embeddings (4096,1024) f32,
negative_modal_embeddings (4096,1024) f32) and returns the reference's
10-tuple. Internally shards the step axis across 8 NeuronCores.

Device math (per core, 1024 step rows):
  raw  = stepT.T @ modalT            (fp32r matmul, PSUM fp32)
  P    = raw * (1/||modal_j||)       = S_ij * ||step_i||
  per-chunk softmax stats with chunk-local maxes, e stored bf16 in SBUF,
  att  = e * exp(M_chunk - M_row)/denom  (device fixup, bf16 out)
  wnum = sum_j P*e, Q = stepT.T @ (neg/||neg||)T -> row max
Host: means/scalars in fp64 from per-row stats; rows whose softmax has
non-negligible mass outside the top entry (fp32r logit noise matters
there) are recomputed exactly in numpy.
"""

import numpy as np

# ---------------- configuration ----------------
N_CORES = 8
N_STEPS = 8192
D = 1024
M = 4096
S_CORE = N_STEPS // N_CORES   # 1024 step rows per core
MC = 1024                     # modal column chunk (streaming granularity)
TEMPERATURE = 0.07
MARGIN = 0.2
EPS = 1e-8
# rows whose attention 2nd-largest entry exceeds this get an exact host redo
SPLIT_ROW_TAU = 3e-4

_CACHE = {}


def _round_fp32r(a: np.ndarray) -> np.ndarray:
    """Round fp32 to the PE's fp32r format (11 explicit mantissa bits)."""
    u = np.ascontiguousarray(a, dtype=np.float32).view(np.uint32)
    r = ((u.astype(np.uint64) + 0x800) & np.uint64(0xFFFFF000)).astype(np.uint32)
    return r.view(np.float32)


def build_kernel(s_core=S_CORE, d=D, m=M, mc=MC, repeat=1):
    import concourse.mybir as mybir
    import concourse.tile as tile
    from concourse import bacc

    F32, F32R, BF16 = mybir.dt.float32, mybir.dt.float32r, mybir.dt.bfloat16
    Alu = mybir.AluOpType
    Act = mybir.ActivationFunctionType
    X = mybir.AxisListType.X

    n_sc = s_core // 128          # step sub-chunks of 128 rows
    n_dc = d // 128               # contraction chunks
    n_mc = m // mc                # modal column chunks
    n_h = mc // 512               # matmuls per (d-chunk, m-chunk)
    inv_t = 1.0 / TEMPERATURE

    nc = bacc.Bacc("TRN2", target_bir_lowering=False, debug=False,
                   num_devices=N_CORES)
    xT = nc.dram_tensor("xT", [d, s_core], F32R, kind="ExternalInput").ap()
    mT = nc.dram_tensor("mT", [d, m], F32R, kind="ExternalInput").ap()
    n2T = nc.dram_tensor("n2T", [d, m], F32R, kind="ExternalInput").ap()
    rmn_rep = nc.dram_tensor("rmn_rep", [128, m], F32, kind="ExternalInput").ap()
    att_b = nc.dram_tensor("att_b", [s_core, m], BF16, kind="ExternalOutput").ap()
    mp_o = nc.dram_tensor("mp_o", [s_core, 1], F32, kind="ExternalOutput").ap()
    wt_o = nc.dram_tensor("wt_o", [s_core, 1], F32, kind="ExternalOutput").ap()
    dn_o = nc.dram_tensor("dn_o", [s_core, 1], F32, kind="ExternalOutput").ap()
    mq_o = nc.dram_tensor("mq_o", [s_core, 1], F32, kind="ExternalOutput").ap()

    with tile.TileContext(nc) as tc:
        import contextlib
        ctx = contextlib.ExitStack()
        with ctx:
            singles = ctx.enter_context(tc.tile_pool(name="singles", bufs=1))
            mpool = ctx.enter_context(tc.tile_pool(name="mpool", bufs=2))
            rmnpool = ctx.enter_context(tc.tile_pool(name="rmnpool", bufs=2))
            ppool = ctx.enter_context(tc.tile_pool(name="ppool", bufs=2))
            pspool = ctx.enter_context(tc.tile_pool(name="ps", bufs=3, space="PSUM"))
            outpool = ctx.enter_context(tc.tile_pool(name="outpool", bufs=3))
            smalls = ctx.enter_context(tc.tile_pool(name="smalls", bufs=8))

            # resident tensors
            xt = singles.tile([128, n_dc, s_core], F32R)
            nc.sync.dma_start(out=xt, in_=xT.rearrange("(c p) s -> p c s", p=128))
            e_tiles = {
                (isc, imc): singles.tile([128, mc], BF16, tag=f"e{isc}_{imc}",
                                         name=f"e{isc}_{imc}")
                for isc in range(n_sc) for imc in range(n_mc)
            }
            # per-(s-chunk, m-chunk) stats
            m_st = singles.tile([128, n_sc, n_mc], F32)   # chunk max of raw
            d_st = singles.tile([128, n_sc, n_mc], F32)   # chunk sum of e
            w_st = singles.tile([128, n_sc, n_mc], F32)   # chunk sum of P*e
            mp_st = singles.tile([128, n_sc, n_mc], F32)  # chunk max of P
            mq_st = singles.tile([128, n_sc, n_mc], F32)  # chunk max of Q

            def body():
                # ---------------- positive phase ----------------
                mT_r = mT.rearrange("(c p) m -> p c m", p=128)
                n2_r = n2T.rearrange("(c p) m -> p c m", p=128)
                for imc in range(n_mc):
                    mt = mpool.tile([128, n_dc, mc], F32R, tag="mt")
                    nc.sync.dma_start(out=mt, in_=mT_r[:, :, imc * mc:(imc + 1) * mc])
                    rmn = rmnpool.tile([128, mc], F32, tag="rmn")
                    nc.sync.dma_start(out=rmn, in_=rmn_rep[:, imc * mc:(imc + 1) * mc])
                    for isc in range(n_sc):
                        raw = pspool.tile([128, mc], F32, tag="acc")
                        lhs = [xt[:, c, isc * 128:(isc + 1) * 128] for c in range(n_dc)]
                        for c in range(n_dc):
                            for h in range(n_h):
                                nc.tensor.matmul(
                                    raw[:, h * 512:(h + 1) * 512], lhs[c],
                                    mt[:, c, h * 512:(h + 1) * 512],
                                    start=(c == 0), stop=(c == n_dc - 1))
                        # row ops on this [128, mc] block
                        nc.vector.reduce_max(m_st[:, isc, imc:imc + 1], raw, axis=X)
                        p_t = ppool.tile([128, mc], F32, tag="p")
                        nc.vector.tensor_mul(p_t, raw, rmn)
                        nc.vector.reduce_max(mp_st[:, isc, imc:imc + 1], p_t, axis=X)
                        negm = smalls.tile([128, 1], F32, tag="negm")
                        nc.vector.tensor_scalar_mul(
                            negm, m_st[:, isc, imc:imc + 1], -inv_t)
                        e_sl = e_tiles[(isc, imc)]
                        nc.scalar.activation(
                            out=e_sl, in_=raw, func=Act.Exp,
                            bias=negm, scale=inv_t,
                            accum_out=d_st[:, isc, imc:imc + 1])
                        # wnum chunk: sum(P*e) ; overwrite p_t in place
                        nc.vector.scalar_tensor_tensor(
                            out=p_t, in0=p_t, scalar=1.0, in1=e_sl,
                            op0=Alu.bypass, op1=Alu.mult,
                            accum_out=w_st[:, isc, imc:imc + 1])

                # ---------------- positive finalize + fixup ----------------
                for isc in range(n_sc):
                    mrow = smalls.tile([128, 1], F32, tag="mrow")
                    nc.vector.reduce_max(mrow, m_st[:, isc, :], axis=X)
                    negmr = smalls.tile([128, 1], F32, tag="negmr")
                    nc.vector.tensor_scalar_mul(negmr, mrow, -inv_t)
                    g = smalls.tile([128, n_mc], F32, tag="g")
                    nc.scalar.activation(out=g, in_=m_st[:, isc, :], func=Act.Exp,
                                         bias=negmr, scale=inv_t)
                    sc4 = smalls.tile([128, n_mc], F32, tag="sc4")
                    dnr = smalls.tile([128, 1], F32, tag="dnr")
                    nc.vector.scalar_tensor_tensor(
                        out=sc4, in0=d_st[:, isc, :], scalar=1.0, in1=g,
                        op0=Alu.bypass, op1=Alu.mult, accum_out=dnr)
                    sc4b = smalls.tile([128, n_mc], F32, tag="sc4b")
                    wnr = smalls.tile([128, 1], F32, tag="wnr")
                    nc.vector.scalar_tensor_tensor(
                        out=sc4b, in0=w_st[:, isc, :], scalar=1.0, in1=g,
                        op0=Alu.bypass, op1=Alu.mult, accum_out=wnr)
                    rden = smalls.tile([128, 1], F32, tag="rden")
                    nc.vector.reciprocal(rden, dnr)
                    wt_t = smalls.tile([128, 1], F32, tag="wt")
                    nc.vector.tensor_mul(wt_t, wnr, rden)
                    f = smalls.tile([128, n_mc], F32, tag="f")
                    nc.vector.tensor_scalar_mul(f, g, rden)
                    mp_t = smalls.tile([128, 1], F32, tag="mp")
                    nc.vector.reduce_max(mp_t, mp_st[:, isc, :], axis=X)
                    sl = slice(isc * 128, (isc + 1) * 128)
                    nc.sync.dma_start(out=mp_o[sl, :], in_=mp_t)
                    nc.sync.dma_start(out=wt_o[sl, :], in_=wt_t)
                    nc.sync.dma_start(out=dn_o[sl, :], in_=dnr)
                    for imc in range(n_mc):
                        att_t = outpool.tile([128, mc], BF16, tag="att")
                        nc.gpsimd.tensor_scalar_mul(
                            att_t, e_tiles[(isc, imc)], f[:, imc:imc + 1])
                        nc.sync.dma_start(
                            out=att_b[sl, imc * mc:(imc + 1) * mc], in_=att_t)

                # ---------------- negative phase ----------------
                for imc in range(n_mc):
                    nt = mpool.tile([128, n_dc, mc], F32R, tag="mt")
                    nc.sync.dma_start(out=nt, in_=n2_r[:, :, imc * mc:(imc + 1) * mc])
                    for isc in range(n_sc):
                        q = pspool.tile([128, mc], F32, tag="acc")
                        lhs = [xt[:, c, isc * 128:(isc + 1) * 128] for c in range(n_dc)]
                        for c in range(n_dc):
                            for h in range(n_h):
                                nc.tensor.matmul(
                                    q[:, h * 512:(h + 1) * 512], lhs[c],
                                    nt[:, c, h * 512:(h + 1) * 512],
                                    start=(c == 0), stop=(c == n_dc - 1))
                        nc.vector.reduce_max(mq_st[:, isc, imc:imc + 1], q, axis=X)
                for isc in range(n_sc):
                    mq_t = smalls.tile([128, 1], F32, tag="mq")
                    nc.vector.reduce_max(mq_t, mq_st[:, isc, :], axis=X)
                    nc.sync.dma_start(
                        out=mq_o[isc * 128:(isc + 1) * 128, :], in_=mq_t)

            if repeat == 1:
                body()
            else:
                with tc.For_i(0, repeat, 1):
                    body()
    nc.compile()
    return nc


def _get_nc(key, **kw):
    if key not in _CACHE:
        _CACHE[key] = build_kernel(**kw)
    return _CACHE[key]


def kernel(step_embeddings, modal_embeddings, negative_modal_embeddings):
    from concourse.bass_utils import run_bass_kernel_spmd

    x = np.ascontiguousarray(step_embeddings, dtype=np.float32)
    mo = np.ascontiguousarray(modal_embeddings, dtype=np.float32)
    ne = np.ascontiguousarray(negative_modal_embeddings, dtype=np.float32)

    # host-side norms (fp64 for stability)
    sn = np.maximum(np.linalg.norm(x.astype(np.float64), axis=1), EPS)
    mn = np.maximum(np.linalg.norm(mo.astype(np.float64), axis=1), EPS)
    nn = np.maximum(np.linalg.norm(ne.astype(np.float64), axis=1), EPS)
    rsn = 1.0 / sn
    rmn = (1.0 / mn).astype(np.float32)

    mT = _round_fp32r(mo.T)
    n2T = _round_fp32r((ne / nn[:, None].astype(np.float32)).T)
    xT = _round_fp32r(x.T)
    rmn_rep = np.ascontiguousarray(np.broadcast_to(rmn, (128, M)))

    nc = _get_nc("main")
    in_maps = []
    for c in range(N_CORES):
        in_maps.append({
            "xT": np.ascontiguousarray(xT[:, c * S_CORE:(c + 1) * S_CORE]),
            "mT": mT, "n2T": n2T, "rmn_rep": rmn_rep,
        })
    res = run_bass_kernel_spmd(nc, in_maps, core_ids=list(range(N_CORES)))

    att = np.empty((N_STEPS, M), dtype=np.float32)
    mp = np.empty(N_STEPS, dtype=np.float64)
    wt = np.empty(N_STEPS, dtype=np.float64)
    mq = np.empty(N_STEPS, dtype=np.float64)
    for c in range(N_CORES):
        r = res.results[c]
        sl = slice(c * S_CORE, (c + 1) * S_CORE)
        att[sl] = r["att_b"].astype(np.float32)
        mp[sl] = r["mp_o"][:, 0].astype(np.float64)
        wt[sl] = r["wt_o"][:, 0].astype(np.float64)
        mq[sl] = r["mq_o"][:, 0].astype(np.float64)

    per_step = mp * rsn          # max_j S_ij
    weighted = wt * rsn          # sum_j S_ij * att_ij
    negmax = mq * rsn            # max_j Sn_ij

    # exact host redo of rows where softmax has non-trivial mass beyond top-1
    # (fp32r logit noise only matters there)
    part = np.partition(att, M - 2, axis=1)
    second = part[:, M - 2]
    rows = np.nonzero(second > SPLIT_ROW_TAU)[0]
    if rows.size:
        raw = x[rows].astype(np.float64) @ mo.astype(np.float64).T
        S = raw * rsn[rows][:, None] * (1.0 / mn)[None, :]
        logits = raw / TEMPERATURE
        logits -= logits.max(axis=1, keepdims=True)
        ex = np.exp(logits)
        a_ex = ex / ex.sum(axis=1, keepdims=True)
        att[rows] = a_ex.astype(np.float32)
        per_step[rows] = S.max(axis=1)
        weighted[rows] = (S * a_ex).sum(axis=1)

    alignment = per_step.mean()
    weighted_alignment = weighted.mean()
    negative_alignment = negmax.mean()
    contrastive = alignment - negative_alignment
    margin = max(contrastive - MARGIN, 0.0)
    min_step = per_step.min()
    overall = 0.7 * weighted_alignment + 0.3 * contrastive

    f32 = np.float32
    return (f32(alignment), f32(weighted_alignment), att, f32(contrastive),
            f32(margin), f32(alignment), f32(negative_alignment),
            per_step.astype(np.float32), f32(min_step), f32(overall))
